# revision 8
# baseline (speedup 1.0000x reference)
"""Trainium2 Bass kernel for nn_Attention_7919919694519.

Multi-head attention (B=2, L=2048, H=16, d=64) with two data-dependent masks:
  - V_len[b] masks HEADS h >= V_len[b]: the reference adds -1e12 to every
    score of those heads, which collapses (in fp32) to a uniform softmax, so
    the masked head's output is mean_k(v) = (mean_k V_seq) @ WV_h  (rank-1).
  - Q_len[b] zeroes output rows q >= Q_len[b].

Strategy (host-visible Q_len/V_len drive the work list):
  - Only unmasked heads with live q rows do real attention. Each unmasked
    head is a "job" needing ceil(Q_len[b]/512) q-chunks (the last chunk
    trimmed to its live rows). Jobs are packed into head-slots dealt across
    8 NeuronCores (SPMD: same NEFF, different data); slots are interleaved
    round-robin. No collectives; host scatters/gathers.
  - The QK weight product is reassociated: S = Q (WQ WK^T/sqrt(d)) K^T, so
    one per-slot projection ktTilde = (WK_h WQ_h^T/sqrt(d)) @ K^T replaces
    both q- and k-projections; score matmuls read the raw q DMA directly.
  - Per chunk on device: scores S^T[k,q] in bank-aligned PSUM lanes, exp on
    ScalarE (PSUM->SBUF bf16, the bottleneck engine), AV accumulation with
    a ones-column appended to v so softmax denominators fall out of the
    same matmuls, then reciprocal (VectorE) + ones-matmul broadcast +
    multiply, single bf16 DMA out in O^T layout (host transposes during
    gather). Emission is software-pipelined across chunk-units with 3-deep
    score-PSUM buffering so ScalarE never starves.
  - Masked-head rank-1 content: device reduces V_seq over k (VectorE) and
    projects through WV/2048; host broadcasts rows (pure output assembly).
"""

import math
import numpy as np
import ml_dtypes

import concourse.tile as tile
from concourse import bacc, mybir
from concourse.bass_utils import run_bass_kernel_spmd
from contextlib import ExitStack

BF16 = ml_dtypes.bfloat16
N_CORES = 8
B_, L_, D_, H_ = 2, 2048, 64, 16
NQ = 512              # max q rows per chunk
KT = 16               # number of 128-row k tiles (L/128)
SPS_FD = 1536         # score-psum slot free dim (3 banks)

_cache = {}


def _per_bank(nq):
    """k-tiles packed per 512-f32 PSUM bank (power of two so chunks always
    fill whole banks; outputs never cross a bank boundary)."""
    pb = 1
    while pb * 2 <= min(16, 512 // nq):
        pb *= 2
    return pb


def _chunk_plan(nq, first=False):
    """k-tiles per score chunk: 3 banks per chunk, 2-deep buffered (6 of 8
    PSUM banks; AV accumulators take the rest). Fewer, larger exp
    instructions amortize ScalarE's fixed access latency; narrow q-widths
    pack several k-tiles per bank to keep exp instruction count low."""
    cl = 3 * _per_bank(nq)
    out = [cl] * (KT // cl)
    if KT % cl:
        out = [KT % cl] + out
    return out


def _unit_order(struct):
    """Round-robin (slot, position) order; index = DRAM row in qt/out."""
    order = []
    max_r = max(len(w) for w in struct)
    for r in range(max_r):
        for s in range(len(struct)):
            if r < len(struct[s]):
                order.append((s, r))
    return order


def _build(struct):
    """Build + compile the SPMD NEFF.

    struct: tuple of per-slot tuples of chunk q-widths, e.g.
    ((512, 512, 512, 128), (512, 512, 256))."""
    nc = bacc.Bacc("TRN2", target_bir_lowering=False, debug=False,
                   num_devices=N_CORES)
    dt = mybir.dt
    S = len(struct)
    # interleave slots round-robin so slot prologues overlap earlier slots'
    # compute and the kernel tail lands on the smallest chunk. unit index u
    # equals its DRAM row in qt/out (host uses the same ordering).
    units = [(s, r == 0, struct[s][r]) for s, r in _unit_order(struct)]
    NU = len(units)

    qt_d = nc.dram_tensor("qt", [NU, 64, NQ], dt.bfloat16, kind="ExternalInput").ap()
    kt_d = nc.dram_tensor("kt", [S, 64, L_], dt.bfloat16, kind="ExternalInput").ap()
    vt_d = nc.dram_tensor("vt", [S, 64, L_], dt.bfloat16, kind="ExternalInput").ap()
    w_d = nc.dram_tensor("w", [S, 64, 128], dt.bfloat16, kind="ExternalInput").ap()
    vtb_d = nc.dram_tensor("vtb", [B_, 64, L_], dt.float32, kind="ExternalInput").ap()
    wvm_d = nc.dram_tensor("wvm", [64, H_ * 64], dt.float32, kind="ExternalInput").ap()
    out_d = nc.dram_tensor("out", [NU, 128, 256], dt.bfloat16, kind="ExternalOutput").ap()
    mo_d = nc.dram_tensor("meanout", [128, 8, B_], dt.float32, kind="ExternalOutput").ap()

    with tile.TileContext(nc) as tc, ExitStack() as ctx:
        sbufs = max(2, S)   # all slots' K/V live concurrently (interleaved)
        inp = ctx.enter_context(tc.tile_pool(name="inp", bufs=sbufs))
        proj = ctx.enter_context(tc.tile_pool(name="proj", bufs=sbufs))
        expp = ctx.enter_context(tc.tile_pool(name="expp", bufs=4))
        ob = ctx.enter_context(tc.tile_pool(name="ob", bufs=4))
        single = ctx.enter_context(tc.tile_pool(name="single", bufs=1))
        ps_s = ctx.enter_context(tc.tile_pool(name="ps_s", bufs=2, space="PSUM"))
        ps_a = ctx.enter_context(tc.tile_pool(name="ps_a", bufs=2, space="PSUM"))

        st = [dict() for _ in range(NU)]
        slot_tiles = {}

        def slot_k_prologue(u):
            # w DMA + tile allocation. The whole QK weight product is folded
            # into the K side: ktTilde = (WK_h WQ_h^T / sqrt(d)) @ K^T once
            # per slot, so per-unit score matmuls read the raw qt DMA with no
            # per-unit projection chain. kt/vt DMAs are issued by slot_kv_dma
            # (after the first unit's qt DMA so the critical path leads the
            # DMA queue); the projection itself runs in slot_kproj.
            s, first, _ = units[u]
            if not first or s in slot_tiles:
                return
            w_sb = inp.tile([64, 128], dt.bfloat16, tag="w", name=f"w{s}")
            nc.gpsimd.dma_start(w_sb[:], w_d[s])
            kt_sb = inp.tile([64, L_], dt.bfloat16, tag="kt", name=f"kt{s}")
            vt_sb = inp.tile([64, L_], dt.bfloat16, tag="vt", name=f"vt{s}")
            slot_tiles[s] = [w_sb, None, None, vt_sb, kt_sb]

        kprojd = set()

        def slot_kproj(u):
            s, first, _ = units[u]
            if not first or s in kprojd:
                return
            kprojd.add(s)
            w_sb, _, _, _, kt_sb = slot_tiles[s]
            ktT = proj.tile([64, L_], dt.bfloat16, tag="ktT", name=f"ktT{s}")
            for j in range(4):
                kps = ps_s.tile([64, 512], dt.float32, tag="ps", name=f"kps{s}_{j}")
                nc.tensor.matmul(kps[:], w_sb[:, 0:64],
                                 kt_sb[:, j * 512:(j + 1) * 512],
                                 start=True, stop=True)
                # all copies on DVE: ScalarE stays dedicated to exp
                nc.vector.tensor_copy(ktT[:, j * 512:(j + 1) * 512], kps[:])
            slot_tiles[s][1] = ktT

        kv_dmad = {}

        def slot_kv_dma(u, phase=2):
            s, first, _ = units[u]
            if not first:
                return
            done = kv_dmad.get(s, 0)
            kt_sb, vt_sb = slot_tiles[s][4], slot_tiles[s][3]
            if done < 1 and phase >= 0:
                nc.sync.dma_start(kt_sb[:, 0:512], kt_d[s][:, 0:512])
                kv_dmad[s] = 1
            if kv_dmad[s] < 2 and phase >= 1:
                nc.sync.dma_start(kt_sb[:, 512:], kt_d[s][:, 512:])
                nc.gpsimd.dma_start(vt_sb[:], vt_d[s])
                kv_dmad[s] = 2

        def slot_v_prologue(u):
            s, first, _ = units[u]
            if not first or slot_tiles[s][2] is not None:
                return
            w_sb, vt_sb = slot_tiles[s][0], slot_tiles[s][3]
            # v projection into [k=128, 16, 65] layout (col 64 = ones)
            v_sb = proj.tile([128, KT, 65], dt.bfloat16, tag="v_sb")
            for half in range(2):
                vps = ps_s.tile([128, 8 * 64], dt.float32, tag="ps")
                for j in range(8):
                    t = half * 8 + j
                    nc.tensor.matmul(vps[:, j * 64:(j + 1) * 64],
                                     vt_sb[:, t * 128:(t + 1) * 128],
                                     w_sb[:, 64:128], start=True, stop=True)
                nc.vector.tensor_copy(
                    v_sb[:, half * 8:(half + 1) * 8, 0:64],
                    vps[:].rearrange("p (t d) -> p t d", t=8))
            nc.vector.memset(v_sb[:, :, 64], 1.0)
            slot_tiles[s][2] = v_sb

        def unit_prologue(u):
            s, _, nq = units[u]
            d = st[u]
            d["init"] = True
            d["s"] = s
            d["chunks"] = _chunk_plan(nq, first=(u == 0))
            d["offs"] = [sum(d["chunks"][:i]) for i in range(len(d["chunks"]) + 1)]
            d["nq"] = nq
            qt_sb = inp.tile([64, nq], dt.bfloat16, tag="qt", name=f"qt{u}")
            nc.sync.dma_start(qt_sb[:], qt_d[u][:, 0:nq])
            d["qTh"] = qt_sb
            d["sps"] = [None] * len(d["chunks"])
            d["ex"] = [None] * len(d["chunks"])

        def s_chunk(u, c):
            d = st[u]
            cl, nq = d["chunks"][c], d["nq"]
            pb = _per_bank(nq)
            nb = (cl + pb - 1) // pb
            sps = ps_s.tile([128, nb, pb, nq], dt.float32, tag="ps",
                            name=f"sps{u}_{c}",
                            padded_shape=[None, None, None, 512 // pb])
            for j in range(cl):
                t = d["offs"][c] + j
                nc.tensor.matmul(sps[:, j // pb, j % pb, :],
                                 slot_tiles[d["s"]][1][:, t * 128:(t + 1) * 128],
                                 d["qTh"][:], start=True, stop=True)
            d["sps"][c] = sps

        def e_chunk(u, c):
            d = st[u]
            cl, nq = d["chunks"][c], d["nq"]
            pb = _per_bank(nq)
            nb = (cl + pb - 1) // pb
            ex = expp.tile([128, nb, pb, nq], dt.bfloat16, tag="ex", name=f"ex{u}_{c}")
            nc.scalar.activation(ex[:], d["sps"][c][:],
                                 mybir.ActivationFunctionType.Exp)
            d["ex"][c] = ex

        def av_chunk(u, c):
            # AV in O[q, d] orientation: lhsT = exp-scores [k, q-subtile],
            # rhs = v_sb [k, 65] (col 64 = ones -> denominators). Output free
            # dim is 65, so PE cost per k-tile is 65*NSUB cycles instead of
            # nq -- about half of the [d, q] orientation for nq=512. All
            # NSUB accumulation regions share one PSUM bank.
            d = st[u]
            nq = d["nq"]
            v_sb = slot_tiles[d["s"]][2]
            nsub = (nq + 127) // 128
            if c == 0:
                d["av"] = ps_a.tile([128, nsub, 65], dt.float32, tag="pa",
                                    name=f"av{u}")
            pb = _per_bank(nq)
            for j in range(d["chunks"][c]):
                t = d["offs"][c] + j
                for s in range(nsub):
                    w = min(128, nq - s * 128)
                    # all NSUB accumulation regions share one PSUM bank; a
                    # start=True matmul zeroes the whole bank, so only the
                    # very first matmul of the unit starts the group and only
                    # the very last stops it.
                    nc.tensor.matmul(
                        d["av"][0:w, s, :],
                        d["ex"][c][:, j // pb, j % pb, s * 128:s * 128 + w],
                        v_sb[:, t, :],
                        start=(t == 0 and s == 0),
                        stop=(t == KT - 1 and s == nsub - 1),
                        skip_group_check=True)

        def epilogue(u):
            # normalize per q-row: reciprocal of the ones-column, then one
            # per-partition tensor_scalar multiply per 128-row subtile.
            # Output lands directly in [q, d] layout (no host transpose).
            d = st[u]
            nq = d["nq"]
            nsub = (nq + 127) // 128
            rcp = ob.tile([128, nsub], dt.float32, tag="rs", name=f"rs{u}")
            ot = ob.tile([128, nsub, 64], dt.bfloat16, tag="ot", name=f"ot{u}")
            for s in range(nsub):
                w = min(128, nq - s * 128)
                nc.vector.reciprocal(rcp[0:w, s:s + 1], d["av"][0:w, s, 64:65])
                with nc.allow_low_precision(reason="final output cast; 2e-2 rel-err budget"):
                    nc.vector.tensor_scalar_mul(ot[0:w, s, :],
                                                d["av"][0:w, s, 0:64],
                                                rcp[0:w, s:s + 1])
            nc.sync.dma_start(out_d[u][:, 0:nsub * 64],
                              ot[:].rearrange("p a b -> p (a b)"))
            st[u].clear()

        def mean_block():
            # masked-head rank-1 content: (sum_k V_seq) @ (WV/2048)
            wvm_sb = single.tile([64, H_ * 64], dt.float32)
            nc.sync.dma_start(wvm_sb[:], wvm_d[:])
            mvt = single.tile([64, B_], dt.float32)
            mvt4 = single.tile([64, B_, 4], dt.float32)
            for b in range(B_):
                vtb_sb = inp.tile([64, L_], dt.float32, tag="vtb")
                nc.sync.dma_start(vtb_sb[:], vtb_d[b])
                for j in range(4):
                    nc.vector.reduce_sum(mvt4[:, b, j:j + 1],
                                         vtb_sb[:, j * 512:(j + 1) * 512],
                                         axis=mybir.AxisListType.X)
                nc.vector.reduce_sum(mvt[:, b:b + 1], mvt4[:, b, :],
                                     axis=mybir.AxisListType.X)
            mo_sb = single.tile([128, 8, B_], dt.float32)
            mps = ps_a.tile([128, 8, B_], dt.float32, tag="pa", name="mps")
            for c in range(8):
                nc.tensor.matmul(mps[:, c, :], wvm_sb[:, c * 128:(c + 1) * 128],
                                 mvt[:], start=True, stop=True)
            nc.vector.tensor_copy(mo_sb[:], mps[:])
            nc.sync.dma_start(mo_d[:], mo_sb[:])

        # software pipeline across chunk-units: the next unit's prologue and
        # first score chunk are emitted before this unit's AV tail/epilogue so
        # ScalarE never starves at unit boundaries.
        slot_k_prologue(0)
        slot_kv_dma(0, phase=0)
        unit_prologue(0)
        slot_kv_dma(0, phase=1)
        slot_kproj(0)
        s_chunk(0, 0)
        e_chunk(0, 0)
        # prefetch every other slot's K/V DMAs + projection while unit 0 runs
        first_unit = {}
        for i, (s, first, _) in enumerate(units):
            if first:
                first_unit[s] = i
        for s in range(1, S):
            slot_k_prologue(first_unit[s])
            slot_kv_dma(first_unit[s])
            slot_kproj(first_unit[s])

        def prefetch_next(u1):
            if u1 >= NU or st[u1].get("init"):
                return
            slot_k_prologue(u1)
            slot_kv_dma(u1)
            slot_kproj(u1)
            unit_prologue(u1)
            s_chunk(u1, 0)
            e_chunk(u1, 0)

        if NU > 1:
            prefetch_next(1)
        for u in range(NU):
            nch = len(st[u]["chunks"])
            for c in range(nch):
                if c + 1 < nch:
                    s_chunk(u, c + 1)
                    e_chunk(u, c + 1)
                    if c == max(0, nch - 2):
                        prefetch_next(u + 1)
                elif u + 1 < NU:
                    prefetch_next(u + 1)
                if c == 0:
                    slot_v_prologue(u)
                av_chunk(u, c)
            epilogue(u)
            if u == max(0, NU // 2 - 1):
                mean_block()

    nc.compile()
    return nc


def _plan(q_len, v_len, B, L, H):
    """Pack unmasked-head jobs into head-slots.

    Returns (struct, assign): struct[s] = tuple of chunk q-widths;
    assign[(core, s)] = (b, h) or None."""
    jobs = []
    for b in range(B):
        nq = min(max(q_len[b], 0), L)
        nh = min(max(v_len[b], 0), H)
        if nq <= 0:
            continue
        r = (nq + NQ - 1) // NQ
        for h in range(nh):
            jobs.append((r, nq, b, h))
    jobs.sort(key=lambda x: (-x[0], -x[1]))
    n_slots = max(1, (len(jobs) + N_CORES - 1) // N_CORES)
    struct = []
    assign = {}
    for s in range(n_slots):
        col = jobs[s * N_CORES:(s + 1) * N_CORES]
        rmax = col[0][0] if col else 1
        widths = []
        for r in range(rmax):
            live = max((min(NQ, nq - r * NQ) for (jr, nq, _, _) in col
                        if r < jr), default=64)
            widths.append(int(live))
        struct.append(tuple(widths))
        for c in range(N_CORES):
            assign[(c, s)] = (col[c][2], col[c][3]) if c < len(col) else None
    return tuple(struct), assign


def kernel(Q_seq, K_seq, V_seq, WQ, WK, WV, Q_len, V_len):
    Q_seq = np.asarray(Q_seq, dtype=np.float32)
    K_seq = np.asarray(K_seq, dtype=np.float32)
    V_seq = np.asarray(V_seq, dtype=np.float32)
    WQ = np.asarray(WQ, dtype=np.float32)
    WK = np.asarray(WK, dtype=np.float32)
    WV = np.asarray(WV, dtype=np.float32)
    q_len = [int(x) for x in np.asarray(Q_len).reshape(-1)]
    v_len = [int(x) for x in np.asarray(V_len).reshape(-1)]
    B, L, d = Q_seq.shape
    H = WQ.shape[1] // d
    scale = 1.0 / math.sqrt(d)

    struct, assign = _plan(q_len, v_len, B, L, H)
    S = len(struct)
    order = _unit_order(struct)
    row_of = {sr: i for i, sr in enumerate(order)}
    NU = len(order)

    if struct not in _cache:
        _cache[struct] = _build(struct)
    nc = _cache[struct]

    # host-side shard prep (transposes, bf16 casts, weight slicing)
    KTb = [np.ascontiguousarray(K_seq[b].T).astype(BF16) for b in range(B)]
    VTb = [np.ascontiguousarray(V_seq[b].T).astype(BF16) for b in range(B)]
    QT = [np.ascontiguousarray(Q_seq[b].T).astype(BF16) for b in range(B)]
    vtb = np.stack([V_seq[b].T for b in range(B)]).astype(np.float32)
    wvm = (WV / float(L)).astype(np.float32)

    in_maps = []
    for c in range(N_CORES):
        qt = np.zeros((NU, 64, NQ), dtype=BF16)
        kt = np.zeros((S, 64, L), dtype=BF16)
        vt = np.zeros((S, 64, L), dtype=BF16)
        w = np.zeros((S, 64, 128), dtype=BF16)
        for s in range(S):
            job = assign[(c, s)]
            if job is None:
                continue
            b, h = job
            kt[s] = KTb[b]
            vt[s] = VTb[b]
            wq_h = WQ[:, h * d:(h + 1) * d]
            wk_h = WK[:, h * d:(h + 1) * d]
            w[s, :, 0:64] = (wk_h @ wq_h.T * scale).astype(BF16)
            w[s, :, 64:128] = WV[:, h * d:(h + 1) * d].astype(BF16)
            for r, nqw in enumerate(struct[s]):
                q0 = min(r * NQ, L - nqw)
                qt[row_of[(s, r)], :, 0:nqw] = QT[b][:, q0:q0 + nqw]
        in_maps.append({"qt": qt, "kt": kt, "vt": vt, "w": w,
                        "vtb": vtb, "wvm": wvm})

    global _last_in_maps
    _last_in_maps = in_maps
    res = run_bass_kernel_spmd(nc, in_maps, core_ids=list(range(N_CORES)))
    results = res.results

    # gather
    out = np.zeros((B, L, H * d), dtype=np.float32)
    mo = results[0]["meanout"]  # [128, 8, B]
    mean_proj = np.transpose(mo, (2, 1, 0)).reshape(B, H * d)  # [B, H*d]
    for b in range(B):
        nq = min(max(q_len[b], 0), L)
        nh = min(max(v_len[b], 0), H)
        if nq > 0 and nh < H:
            out[b, :nq, nh * d:] = mean_proj[b, nh * d:][None, :]
    for (c, s), job in assign.items():
        if job is None:
            continue
        b, h = job
        nq = min(max(q_len[b], 0), L)
        for r, nqw in enumerate(struct[s]):
            q0 = min(r * NQ, L - nqw)
            blk = results[c]["out"][row_of[(s, r)]].reshape(128, 4, 64)
            for sub in range((nqw + 127) // 128):
                w = min(128, nqw - sub * 128)
                lo = q0 + sub * 128
                hi = min(lo + w, nq)
                if hi <= lo:
                    continue
                out[b, lo:hi, h * d:(h + 1) * d] = \
                    blk[0:hi - lo, sub, :].astype(np.float32)
    return out



# revision 18
# speedup vs baseline: 1.0115x; 1.0115x over previous
"""Trainium2 Bass kernel for nn_Attention_7919919694519.

Multi-head attention (B=2, L=2048, H=16, d=64) with two data-dependent masks:
  - V_len[b] masks HEADS h >= V_len[b]: the reference adds -1e12 to every
    score of those heads, which collapses (in fp32) to a uniform softmax, so
    the masked head's output is mean_k(v) = (mean_k V_seq) @ WV_h  (rank-1).
  - Q_len[b] zeroes output rows q >= Q_len[b].

Strategy (host-visible Q_len/V_len drive the work list):
  - Only unmasked heads with live q rows do real attention. Each unmasked
    head is a "job" needing ceil(Q_len[b]/512) q-chunks (the last chunk
    trimmed to its live rows). Jobs are packed into head-slots dealt across
    8 NeuronCores (SPMD: same NEFF, different data); slots are interleaved
    round-robin. No collectives; host scatters/gathers.
  - The QK weight product is reassociated: S = Q (WQ WK^T/sqrt(d)) K^T, so
    one per-slot projection ktTilde = (WK_h WQ_h^T/sqrt(d)) @ K^T replaces
    both q- and k-projections; score matmuls read the raw q DMA directly.
  - Per chunk on device: scores S^T[k,q] in bank-aligned PSUM lanes, exp on
    ScalarE (PSUM->SBUF bf16, the bottleneck engine), AV accumulation with
    a ones-column appended to v so softmax denominators fall out of the
    same matmuls, then reciprocal (VectorE) + ones-matmul broadcast +
    multiply, single bf16 DMA out in O^T layout (host transposes during
    gather). Emission is software-pipelined across chunk-units with 3-deep
    score-PSUM buffering so ScalarE never starves.
  - Masked-head rank-1 content: device reduces V_seq over k (VectorE) and
    projects through WV/2048; host broadcasts rows (pure output assembly).
"""

import math
import numpy as np
import ml_dtypes

import concourse.tile as tile
from concourse import bacc, mybir
from concourse.bass_utils import run_bass_kernel_spmd
from contextlib import ExitStack

BF16 = ml_dtypes.bfloat16
N_CORES = 8
B_, L_, D_, H_ = 2, 2048, 64, 16
NQ = 512              # max q rows per chunk
KT = 16               # number of 128-row k tiles (L/128)
SPS_FD = 1536         # score-psum slot free dim (3 banks)

_cache = {}


def _per_bank(nq):
    """k-tiles packed per 512-f32 PSUM bank (power of two so chunks always
    fill whole banks; outputs never cross a bank boundary)."""
    pb = 1
    while pb * 2 <= min(16, 512 // nq):
        pb *= 2
    return pb


def _chunk_plan(nq, first=False):
    """k-tiles per score chunk: 3 banks per chunk, 2-deep buffered (6 of 8
    PSUM banks; AV accumulators take the rest). Fewer, larger exp
    instructions amortize ScalarE's fixed access latency; narrow q-widths
    pack several k-tiles per bank to keep exp instruction count low.

    The ragged chunk goes first on unit 0 (prime ScalarE as early as
    possible) and last elsewhere (small kernel tail)."""
    cl = 3 * _per_bank(nq)
    out = [cl] * (KT // cl)
    if KT % cl:
        out = [KT % cl] + out if first else out + [KT % cl]
    return out


def _unit_order(struct):
    """Round-robin (slot, position) order; index = DRAM row in qt/out."""
    order = []
    max_r = max(len(w) for w in struct)
    for r in range(max_r):
        for s in range(len(struct)):
            if r < len(struct[s]):
                order.append((s, r))
    return order


def _build(struct):
    """Build + compile the SPMD NEFF.

    struct: tuple of per-slot tuples of chunk q-widths, e.g.
    ((512, 512, 512, 128), (512, 512, 256))."""
    nc = bacc.Bacc("TRN2", target_bir_lowering=False, debug=False,
                   num_devices=N_CORES)
    dt = mybir.dt
    S = len(struct)
    # interleave slots round-robin so slot prologues overlap earlier slots'
    # compute and the kernel tail lands on the smallest chunk. unit index u
    # equals its DRAM row in qt/out (host uses the same ordering).
    units = [(s, r == 0, struct[s][r]) for s, r in _unit_order(struct)]
    NU = len(units)

    qt_d = nc.dram_tensor("qt", [NU, 64, NQ], dt.bfloat16, kind="ExternalInput").ap()
    kt_d = nc.dram_tensor("kt", [S, 64, L_], dt.bfloat16, kind="ExternalInput").ap()
    vt_d = nc.dram_tensor("vt", [S, 64, L_], dt.bfloat16, kind="ExternalInput").ap()
    w_d = nc.dram_tensor("w", [S, 64, 128], dt.bfloat16, kind="ExternalInput").ap()
    vkm_d = nc.dram_tensor("vkm", [B_, 128, KT * 64], dt.bfloat16, kind="ExternalInput").ap()
    wvm_d = nc.dram_tensor("wvm", [64, H_ * 64], dt.float32, kind="ExternalInput").ap()
    out_d = nc.dram_tensor("out", [NU, 128, 256], dt.bfloat16, kind="ExternalOutput").ap()
    mo_d = nc.dram_tensor("meanout", [128, 8, B_], dt.float32, kind="ExternalOutput").ap()

    with tile.TileContext(nc) as tc, ExitStack() as ctx:
        sbufs = max(2, S)   # all slots' K/V live concurrently (interleaved)
        inp = ctx.enter_context(tc.tile_pool(name="inp", bufs=sbufs))
        proj = ctx.enter_context(tc.tile_pool(name="proj", bufs=sbufs))
        expp = ctx.enter_context(tc.tile_pool(name="expp", bufs=4))
        ob = ctx.enter_context(tc.tile_pool(name="ob", bufs=4))
        single = ctx.enter_context(tc.tile_pool(name="single", bufs=1))
        ps_s = ctx.enter_context(tc.tile_pool(name="ps_s", bufs=2, space="PSUM"))
        ps_a = ctx.enter_context(tc.tile_pool(name="ps_a", bufs=2, space="PSUM"))

        st = [dict() for _ in range(NU)]
        slot_tiles = {}

        def slot_k_prologue(u):
            # w DMA + tile allocation. The whole QK weight product is folded
            # into the K side: ktTilde = (WK_h WQ_h^T / sqrt(d)) @ K^T once
            # per slot, so per-unit score matmuls read the raw qt DMA with no
            # per-unit projection chain. kt/vt DMAs are issued by slot_kv_dma
            # (after the first unit's qt DMA so the critical path leads the
            # DMA queue); the projection itself runs in slot_kproj.
            s, first, _ = units[u]
            if not first or s in slot_tiles:
                return
            w_sb = inp.tile([64, 128], dt.bfloat16, tag="w", name=f"w{s}")
            # sync queue: the K-side weight product gates the whole exp chain
            nc.sync.dma_start(w_sb[:], w_d[s])
            kt_sb = inp.tile([64, L_], dt.bfloat16, tag="kt", name=f"kt{s}")
            vt_sb = inp.tile([64, L_], dt.bfloat16, tag="vt", name=f"vt{s}")
            slot_tiles[s] = [w_sb, None, None, vt_sb, kt_sb]

        kprojd = set()

        def slot_kproj(u):
            s, first, _ = units[u]
            if not first or s in kprojd:
                return
            kprojd.add(s)
            w_sb, _, _, _, kt_sb = slot_tiles[s]
            ktT = proj.tile([64, L_], dt.bfloat16, tag="ktT", name=f"ktT{s}")
            for j in range(4):
                kps = ps_s.tile([64, 512], dt.float32, tag="ps", name=f"kps{s}_{j}")
                nc.tensor.matmul(kps[:], w_sb[:, 0:64],
                                 kt_sb[:, j * 512:(j + 1) * 512],
                                 start=True, stop=True)
                # all copies on DVE: ScalarE stays dedicated to exp
                nc.vector.tensor_copy(ktT[:, j * 512:(j + 1) * 512], kps[:])
            slot_tiles[s][1] = ktT

        kv_dmad = {}

        def slot_kv_dma(u, phase=2):
            s, first, _ = units[u]
            if not first:
                return
            done = kv_dmad.get(s, 0)
            kt_sb, vt_sb = slot_tiles[s][4], slot_tiles[s][3]
            if done < 1 and phase >= 0:
                nc.sync.dma_start(kt_sb[:, 0:512], kt_d[s][:, 0:512])
                kv_dmad[s] = 1
            if kv_dmad[s] < 2 and phase >= 1:
                nc.sync.dma_start(kt_sb[:, 512:], kt_d[s][:, 512:])
                nc.gpsimd.dma_start(vt_sb[:], vt_d[s])
                kv_dmad[s] = 2

        def slot_v_prologue(u):
            s, first, _ = units[u]
            if not first or slot_tiles[s][2] is not None:
                return
            w_sb, vt_sb = slot_tiles[s][0], slot_tiles[s][3]
            # v projection into [k=128, 16, 65] layout (col 64 = ones)
            v_sb = proj.tile([128, KT, 65], dt.bfloat16, tag="v_sb")
            for half in range(2):
                vps = ps_s.tile([128, 8 * 64], dt.float32, tag="ps")
                for j in range(8):
                    t = half * 8 + j
                    nc.tensor.matmul(vps[:, j * 64:(j + 1) * 64],
                                     vt_sb[:, t * 128:(t + 1) * 128],
                                     w_sb[:, 64:128], start=True, stop=True)
                nc.vector.tensor_copy(
                    v_sb[:, half * 8:(half + 1) * 8, 0:64],
                    vps[:].rearrange("p (t d) -> p t d", t=8))
            nc.vector.memset(v_sb[:, :, 64], 1.0)
            slot_tiles[s][2] = v_sb

        def unit_prologue(u):
            s, _, nq = units[u]
            d = st[u]
            d["init"] = True
            d["s"] = s
            d["chunks"] = _chunk_plan(nq, first=(u == 0))
            d["offs"] = [sum(d["chunks"][:i]) for i in range(len(d["chunks"]) + 1)]
            d["nq"] = nq
            qt_sb = inp.tile([64, nq], dt.bfloat16, tag="qt", name=f"qt{u}")
            nc.sync.dma_start(qt_sb[:], qt_d[u][:, 0:nq])
            d["qTh"] = qt_sb
            d["sps"] = [None] * len(d["chunks"])
            d["ex"] = [None] * len(d["chunks"])
            d["next_c"] = 0

        def s_chunk(u, c):
            d = st[u]
            cl, nq = d["chunks"][c], d["nq"]
            pb = _per_bank(nq)
            nb = (cl + pb - 1) // pb
            sps = ps_s.tile([128, nb, pb, nq], dt.float32, tag="ps",
                            name=f"sps{u}_{c}",
                            padded_shape=[None, None, None, 512 // pb])
            for j in range(cl):
                t = d["offs"][c] + j
                nc.tensor.matmul(sps[:, j // pb, j % pb, :],
                                 slot_tiles[d["s"]][1][:, t * 128:(t + 1) * 128],
                                 d["qTh"][:], start=True, stop=True)
            d["sps"][c] = sps

        def e_chunk(u, c):
            d = st[u]
            cl, nq = d["chunks"][c], d["nq"]
            pb = _per_bank(nq)
            nb = (cl + pb - 1) // pb
            ex = expp.tile([128, nb, pb, nq], dt.bfloat16, tag="ex", name=f"ex{u}_{c}")
            nc.scalar.activation(ex[:], d["sps"][c][:],
                                 mybir.ActivationFunctionType.Exp)
            d["ex"][c] = ex

        def av_chunk(u, c):
            # AV in O[q, d] orientation: lhsT = exp-scores [k, q-subtile],
            # rhs = v_sb [k, 65] (col 64 = ones -> denominators). Output free
            # dim is 65, so PE cost per k-tile is 65*NSUB cycles instead of
            # nq -- about half of the [d, q] orientation for nq=512. All
            # NSUB accumulation regions share one PSUM bank.
            d = st[u]
            nq = d["nq"]
            v_sb = slot_tiles[d["s"]][2]
            nsub = (nq + 127) // 128
            if c == 0:
                d["av"] = ps_a.tile([128, nsub, 65], dt.float32, tag="pa",
                                    name=f"av{u}")
            pb = _per_bank(nq)
            for j in range(d["chunks"][c]):
                t = d["offs"][c] + j
                for s in range(nsub):
                    w = min(128, nq - s * 128)
                    # all NSUB accumulation regions share one PSUM bank; a
                    # start=True matmul zeroes the whole bank, so only the
                    # very first matmul of the unit starts the group and only
                    # the very last stops it.
                    nc.tensor.matmul(
                        d["av"][0:w, s, :],
                        d["ex"][c][:, j // pb, j % pb, s * 128:s * 128 + w],
                        v_sb[:, t, :],
                        start=(t == 0 and s == 0),
                        stop=(t == KT - 1 and s == nsub - 1),
                        skip_group_check=True)

        def epilogue(u):
            # normalize per q-row: reciprocal of the ones-column, then one
            # per-partition tensor_scalar multiply per 128-row subtile.
            # Output lands directly in [q, d] layout (no host transpose).
            d = st[u]
            nq = d["nq"]
            nsub = (nq + 127) // 128
            rcp = ob.tile([128, nsub], dt.float32, tag="rs", name=f"rs{u}")
            ot = ob.tile([128, nsub, 64], dt.bfloat16, tag="ot", name=f"ot{u}")
            for s in range(nsub):
                w = min(128, nq - s * 128)
                nc.vector.reciprocal(rcp[0:w, s:s + 1], d["av"][0:w, s, 64:65])
                with nc.allow_low_precision(reason="final output cast; 2e-2 rel-err budget"):
                    nc.vector.tensor_scalar_mul(ot[0:w, s, :],
                                                d["av"][0:w, s, 0:64],
                                                rcp[0:w, s:s + 1])
            nc.sync.dma_start(out_d[u][:, 0:nsub * 64],
                              ot[:].rearrange("p a b -> p (a b)"))
            st[u].clear()

        def mean_block():
            # masked-head rank-1 content: (sum_k V_seq) @ (WV/2048). The
            # k-sum runs on PE (V in k-major layout x ones column) so DVE
            # stays free for the pipeline's copies and epilogues.
            wvm_sb = single.tile([64, H_ * 64], dt.float32)
            nc.sync.dma_start(wvm_sb[:], wvm_d[:])
            ones1 = single.tile([128, 1], dt.bfloat16)
            nc.vector.memset(ones1[:], 1.0)
            mvp = ps_a.tile([64, B_], dt.float32, tag="pa", name="mvp")
            vkm_sb = single.tile([128, B_, KT, 64], dt.bfloat16)
            for b in range(B_):
                nc.sync.dma_start(vkm_sb[:, b], vkm_d[b])
            for b in range(B_):
                for t in range(KT):
                    nc.tensor.matmul(mvp[:, b:b + 1], vkm_sb[:, b, t, :],
                                     ones1[:],
                                     start=(b == 0 and t == 0),
                                     stop=(b == B_ - 1 and t == KT - 1),
                                     skip_group_check=True)
            mvt = single.tile([64, B_], dt.float32)
            nc.vector.tensor_copy(mvt[:], mvp[:])
            mo_sb = single.tile([128, 8, B_], dt.float32)
            mps = ps_a.tile([128, 8, B_], dt.float32, tag="pa", name="mps")
            for c in range(8):
                nc.tensor.matmul(mps[:, c, :], wvm_sb[:, c * 128:(c + 1) * 128],
                                 mvt[:], start=(c == 0), stop=(c == 7),
                                 skip_group_check=True)
            nc.vector.tensor_copy(mo_sb[:], mps[:])
            nc.sync.dma_start(mo_d[:], mo_sb[:])

        # software pipeline across chunk-units: the next unit's prologue and
        # first TWO score chunks are emitted before this unit's AV tail and
        # epilogue so ScalarE never starves at unit boundaries.
        def emit_se(u1, c):
            if u1 >= NU or not st[u1].get("init"):
                return
            d = st[u1]
            if c >= len(d["chunks"]) or c < d["next_c"]:
                return
            s_chunk(u1, c)
            e_chunk(u1, c)
            d["next_c"] = c + 1

        slot_k_prologue(0)
        slot_kv_dma(0, phase=0)
        unit_prologue(0)
        slot_kv_dma(0, phase=1)
        slot_kproj(0)
        emit_se(0, 0)
        # prefetch every other slot's K/V DMAs + projection while unit 0 runs
        first_unit = {}
        for i, (s, first, _) in enumerate(units):
            if first:
                first_unit[s] = i
        for s in range(1, S):
            slot_k_prologue(first_unit[s])
            slot_kv_dma(first_unit[s])
            slot_kproj(first_unit[s])

        def prefetch_next(u1):
            if u1 >= NU or st[u1].get("init"):
                return
            slot_k_prologue(u1)
            slot_kv_dma(u1)
            slot_kproj(u1)
            unit_prologue(u1)
            emit_se(u1, 0)

        if NU > 1:
            prefetch_next(1)
        for u in range(NU):
            nch = len(st[u]["chunks"])
            for c in range(nch):
                emit_se(u, c + 1)
                if c == max(0, nch - 2):
                    prefetch_next(u + 1)
                if c == nch - 1:
                    emit_se(u + 1, 1)
                if c == 0:
                    slot_v_prologue(u)
                av_chunk(u, c)
            epilogue(u)
            if u == max(0, NU // 2 - 1):
                mean_block()

    nc.compile()
    return nc


def _units_of(rows):
    out = []
    while rows > NQ:
        out.append(NQ)
        rows -= NQ
    out.append(int(rows))
    return tuple(out)


def _act_cost(R):
    """ScalarE-time proxy for a slot-size vector: exp elements + per-instr
    access latency + per-slot misc."""
    t = 0.0
    for rows in R:
        for w in _units_of(rows):
            cl = 3 * _per_bank(w)
            t += 16 * w * 0.8333 + 190 * ((KT + cl - 1) // cl)
        t += 200.0
    return t


def _cover(needs, R, limit=60000):
    """Cover each job (needs, descending) with pieces drawn from 8 instances
    of each slot size R[j]. Returns per-job lists of slot indices, or None."""
    J = len(R)
    nodes = [0]

    def combos(need, avail):
        idxs = [j for j in range(J) if avail[j] > 0]
        out = []
        for k in range(1, 5):
            for ms in itertools.combinations_with_replacement(idxs, k):
                cnt = {}
                ok = True
                for j in ms:
                    cnt[j] = cnt.get(j, 0) + 1
                    if cnt[j] > avail[j]:
                        ok = False
                        break
                if not ok:
                    continue
                ssum = sum(R[j] for j in ms)
                if ssum >= need:
                    out.append((ssum - need, k, ms))
        out.sort(key=lambda x: (x[0], x[1]))
        seen, res = set(), []
        for waste, k, ms in out:
            key = tuple(sorted(R[j] for j in ms))
            if key in seen:
                continue
            seen.add(key)
            res.append(ms)
            if len(res) >= 6:
                break
        return res

    def rec(i, avail):
        nodes[0] += 1
        if nodes[0] > limit:
            return None
        if i == len(needs):
            return []
        for ms in combos(needs[i], avail):
            av2 = list(avail)
            for j in ms:
                av2[j] -= 1
            sub = rec(i + 1, av2)
            if sub is not None:
                return [list(ms)] + sub
        return None

    return rec(0, [N_CORES] * J)


def _plan(q_len, v_len, B, L, H):
    """Pack unmasked-head work into uniform per-core slots, splitting heads
    across cores to balance rows (the exp on ScalarE scales with per-core
    rows).

    Returns (struct, assign): struct[s] = tuple of unit q-widths for slot s;
    assign[(core, s)] = (b, h, off) or None, where the piece covers rows
    [off, off + sum(struct[s])) of head (b, h)."""
    jobs = []
    for b in range(B):
        nq = min(max(q_len[b], 0), L)
        nh = min(max(v_len[b], 0), H)
        if nq <= 0:
            continue
        for h in range(nh):
            jobs.append((nq, b, h))
    if not jobs:
        jobs = [(64, 0, 0)]
    jobs.sort(key=lambda x: -x[0])
    needs = [j[0] for j in jobs]

    # guaranteed-feasible fallback: deal whole heads into columns
    n_slots = max(1, (len(jobs) + N_CORES - 1) // N_CORES)
    cands = [tuple(needs[s * N_CORES] for s in range(n_slots))]
    # two-piece/LP family: (B, ceil(A/2), s2, s3) — splits the largest heads
    # in half and covers the remainders with two small slot sizes
    uniq = sorted(set(needs), reverse=True)
    if len(jobs) <= 24 and len(uniq) >= 1:
        A = uniq[0]
        halfA = -(-A // 2)
        for Bn in (uniq[1:2] or [A]):
            for s3 in range(96, min(513, Bn)):
                s2a = -(-(Bn - 3 * s3) // 2)
                s2b = -(-(A - halfA - s3) // 2)
                s2 = max(s2a, s2b, s3, 96)
                if s2 <= 512:
                    cands.append((Bn, halfA, s2, s3))
            cands.append((Bn, halfA, halfA))
            cands.append((A, Bn, 512))
    cands.sort(key=_act_cost)

    best = None
    for R in cands:
        if best is not None and _act_cost(R) >= best[0]:
            continue
        cover = _cover(needs, R)
        if cover is not None:
            best = (_act_cost(R), R, cover)
    _, R, cover = best

    # order slots descending so the round-robin tail lands on a small unit
    order = sorted(range(len(R)), key=lambda j: -R[j])
    inv = {j: i for i, j in enumerate(order)}
    slot_pieces = [[] for _ in range(len(R))]
    for (nq, b, h), ms in zip(jobs, cover):
        acc = 0
        for j in sorted(ms, key=lambda j: -R[j]):
            off = max(0, min(acc, L - R[j]))
            slot_pieces[inv[j]].append((b, h, int(off)))
            acc += R[j]
    struct = tuple(_units_of(R[j]) for j in order)
    assign = {}
    for s in range(len(R)):
        for c in range(N_CORES):
            assign[(c, s)] = (slot_pieces[s][c]
                              if c < len(slot_pieces[s]) else None)
    return struct, assign


def kernel(Q_seq, K_seq, V_seq, WQ, WK, WV, Q_len, V_len):
    Q_seq = np.asarray(Q_seq, dtype=np.float32)
    K_seq = np.asarray(K_seq, dtype=np.float32)
    V_seq = np.asarray(V_seq, dtype=np.float32)
    WQ = np.asarray(WQ, dtype=np.float32)
    WK = np.asarray(WK, dtype=np.float32)
    WV = np.asarray(WV, dtype=np.float32)
    q_len = [int(x) for x in np.asarray(Q_len).reshape(-1)]
    v_len = [int(x) for x in np.asarray(V_len).reshape(-1)]
    B, L, d = Q_seq.shape
    H = WQ.shape[1] // d
    scale = 1.0 / math.sqrt(d)

    struct, assign = _plan(q_len, v_len, B, L, H)
    S = len(struct)
    order = _unit_order(struct)
    row_of = {sr: i for i, sr in enumerate(order)}
    NU = len(order)

    if struct not in _cache:
        _cache[struct] = _build(struct)
    nc = _cache[struct]

    # host-side shard prep (transposes, bf16 casts, weight slicing)
    KTb = [np.ascontiguousarray(K_seq[b].T).astype(BF16) for b in range(B)]
    VTb = [np.ascontiguousarray(V_seq[b].T).astype(BF16) for b in range(B)]
    QT = [np.ascontiguousarray(Q_seq[b].T).astype(BF16) for b in range(B)]
    vkm = np.ascontiguousarray(
        V_seq.reshape(B, KT, 128, d).transpose(0, 2, 1, 3)
    ).reshape(B, 128, KT * d).astype(BF16)
    wvm = (WV / float(L)).astype(np.float32)

    in_maps = []
    for c in range(N_CORES):
        qt = np.zeros((NU, 64, NQ), dtype=BF16)
        kt = np.zeros((S, 64, L), dtype=BF16)
        vt = np.zeros((S, 64, L), dtype=BF16)
        w = np.zeros((S, 64, 128), dtype=BF16)
        for s in range(S):
            job = assign[(c, s)]
            if job is None:
                continue
            b, h = job
            kt[s] = KTb[b]
            vt[s] = VTb[b]
            wq_h = WQ[:, h * d:(h + 1) * d]
            wk_h = WK[:, h * d:(h + 1) * d]
            w[s, :, 0:64] = (wk_h @ wq_h.T * scale).astype(BF16)
            w[s, :, 64:128] = WV[:, h * d:(h + 1) * d].astype(BF16)
            for r, nqw in enumerate(struct[s]):
                q0 = min(r * NQ, L - nqw)
                qt[row_of[(s, r)], :, 0:nqw] = QT[b][:, q0:q0 + nqw]
        in_maps.append({"qt": qt, "kt": kt, "vt": vt, "w": w,
                        "vkm": vkm, "wvm": wvm})

    global _last_in_maps
    _last_in_maps = in_maps
    res = run_bass_kernel_spmd(nc, in_maps, core_ids=list(range(N_CORES)))
    results = res.results

    # gather
    out = np.zeros((B, L, H * d), dtype=np.float32)
    mo = results[0]["meanout"]  # [128, 8, B]
    mean_proj = np.transpose(mo, (2, 1, 0)).reshape(B, H * d)  # [B, H*d]
    for b in range(B):
        nq = min(max(q_len[b], 0), L)
        nh = min(max(v_len[b], 0), H)
        if nq > 0 and nh < H:
            out[b, :nq, nh * d:] = mean_proj[b, nh * d:][None, :]
    for (c, s), job in assign.items():
        if job is None:
            continue
        b, h = job
        nq = min(max(q_len[b], 0), L)
        for r, nqw in enumerate(struct[s]):
            q0 = min(r * NQ, L - nqw)
            blk = results[c]["out"][row_of[(s, r)]].reshape(128, 4, 64)
            for sub in range((nqw + 127) // 128):
                w = min(128, nqw - sub * 128)
                lo = q0 + sub * 128
                hi = min(lo + w, nq)
                if hi <= lo:
                    continue
                out[b, lo:hi, h * d:(h + 1) * d] = \
                    blk[0:hi - lo, sub, :].astype(np.float32)
    return out



# revision 41
# speedup vs baseline: 1.1481x; 1.1351x over previous
"""Trainium2 Bass kernel for nn_Attention_7919919694519.

Multi-head attention (B=2, L=2048, H=16, d=64) with two data-dependent masks:
  - V_len[b] masks HEADS h >= V_len[b]: the reference adds -1e12 to every
    score of those heads, which collapses (in fp32) to a uniform softmax, so
    the masked head's output is mean_k(v) = (mean_k V_seq) @ WV_h  (rank-1).
  - Q_len[b] zeroes output rows q >= Q_len[b].

Strategy (host-visible Q_len/V_len drive the work list):
  - Only unmasked heads with live q rows do real attention. Each unmasked
    head is a "job" needing ceil(Q_len[b]/512) q-chunks (the last chunk
    trimmed to its live rows). Jobs are packed into head-slots dealt across
    8 NeuronCores (SPMD: same NEFF, different data); slots are interleaved
    round-robin. No collectives; host scatters/gathers.
  - The QK weight product is reassociated: S = Q (WQ WK^T/sqrt(d)) K^T, so
    one per-slot projection ktTilde = (WK_h WQ_h^T/sqrt(d)) @ K^T replaces
    both q- and k-projections; score matmuls read the raw q DMA directly.
  - Per chunk on device: scores S^T[k,q] in bank-aligned PSUM lanes, exp on
    ScalarE (PSUM->SBUF bf16, the bottleneck engine), AV accumulation with
    a ones-column appended to v so softmax denominators fall out of the
    same matmuls, then reciprocal (VectorE) + ones-matmul broadcast +
    multiply, single bf16 DMA out in O^T layout (host transposes during
    gather). Emission is software-pipelined across chunk-units with 3-deep
    score-PSUM buffering so ScalarE never starves.
  - Masked-head rank-1 content: device reduces V_seq over k (VectorE) and
    projects through WV/2048; host broadcasts rows (pure output assembly).
"""

import itertools
import math
import numpy as np
import ml_dtypes

import concourse.tile as tile
from concourse import bacc, mybir
from concourse.bass_utils import run_bass_kernel_spmd
from contextlib import ExitStack

BF16 = ml_dtypes.bfloat16
N_CORES = 8
B_, L_, D_, H_ = 2, 2048, 64, 16
NQ = 512              # max q rows per chunk
KT = 16               # number of 128-row k tiles (L/128)
SPS_FD = 1536         # score-psum slot free dim (3 banks)

_cache = {}


def _per_bank(nq):
    """k-tiles packed per 512-f32 PSUM bank (power of two so chunks always
    fill whole banks; outputs never cross a bank boundary)."""
    pb = 1
    while pb * 2 <= min(16, 512 // nq):
        pb *= 2
    return pb


def _chunk_plan(nq, first=False):
    """k-tiles per score chunk: 2 banks per chunk, 3-deep buffered (6 of 8
    PSUM banks; AV accumulators take the rest two). The 3-deep ring keeps PE
    two chunks ahead of ScalarE so semaphore latency never serializes
    exp -> scores -> exp; narrow q-widths pack several k-tiles per bank to
    keep exp instruction count low.

    A ragged chunk goes first on unit 0 (prime ScalarE as early as
    possible) and last elsewhere (small kernel tail)."""
    cl = 3 * _per_bank(nq)
    out = [cl] * (KT // cl)
    if KT % cl:
        out = [KT % cl] + out if first else out + [KT % cl]
    return out


def _unit_order(struct):
    """Round-robin (slot, position) order; index = DRAM row in qt/out."""
    order = []
    max_r = max(len(w) for w in struct)
    for r in range(max_r):
        for s in range(len(struct)):
            if r < len(struct[s]):
                order.append((s, r))
    return order


def _build(struct):
    """Build + compile the SPMD NEFF.

    struct: tuple of per-slot tuples of chunk q-widths, e.g.
    ((512, 512, 512, 128), (512, 512, 256))."""
    nc = bacc.Bacc("TRN2", target_bir_lowering=False, debug=False,
                   num_devices=N_CORES)
    dt = mybir.dt
    S = len(struct)
    # interleave slots round-robin so slot prologues overlap earlier slots'
    # compute and the kernel tail lands on the smallest chunk. unit index u
    # equals its DRAM row in qt/out (host uses the same ordering).
    units = [(s, r == 0, struct[s][r]) for s, r in _unit_order(struct)]
    NU = len(units)

    qt_d = nc.dram_tensor("qt", [NU, 64, NQ], dt.bfloat16, kind="ExternalInput").ap()
    kt_d = nc.dram_tensor("kt", [S, 64, L_], dt.bfloat16, kind="ExternalInput").ap()
    vt_d = nc.dram_tensor("vt", [S, 64, L_], dt.bfloat16, kind="ExternalInput").ap()
    w_d = nc.dram_tensor("w", [S, 64, 128], dt.bfloat16, kind="ExternalInput").ap()
    vkm_d = nc.dram_tensor("vkm", [B_, 128, KT * 64], dt.bfloat16, kind="ExternalInput").ap()
    wvm_d = nc.dram_tensor("wvm", [64, H_ * 64], dt.float32, kind="ExternalInput").ap()
    out_d = nc.dram_tensor("out", [NU, 128, 256], dt.bfloat16, kind="ExternalOutput").ap()
    mo_d = nc.dram_tensor("meanout", [128, 8, B_], dt.float32, kind="ExternalOutput").ap()

    with tile.TileContext(nc) as tc, ExitStack() as ctx:
        sbufs = max(2, S)   # all slots' K/V live concurrently (interleaved)
        inp = ctx.enter_context(tc.tile_pool(name="inp", bufs=sbufs))
        proj = ctx.enter_context(tc.tile_pool(name="proj", bufs=sbufs))
        expp = ctx.enter_context(tc.tile_pool(name="expp", bufs=4))
        ob = ctx.enter_context(tc.tile_pool(name="ob", bufs=4))
        single = ctx.enter_context(tc.tile_pool(name="single", bufs=1))
        ps_s = ctx.enter_context(tc.tile_pool(name="ps_s", bufs=2, space="PSUM"))
        ps_a = ctx.enter_context(tc.tile_pool(name="ps_a", bufs=1, space="PSUM"))
        ps_p = ctx.enter_context(tc.tile_pool(name="ps_p", bufs=1, space="PSUM"))

        st = [dict() for _ in range(NU)]
        slot_tiles = {}
        kv_dmad = {}

        def slot_k_prologue(u):
            # w DMA + tile allocation. The whole QK weight product is folded
            # into the K side: ktTilde = (WK_h WQ_h^T / sqrt(d)) @ K^T once
            # per slot, so per-unit score matmuls read the raw qt DMA with no
            # per-unit projection chain. kt/vt DMAs are issued by slot_kv_dma
            # (after the first unit's qt DMA so the critical path leads the
            # DMA queue); the projection itself runs in slot_kproj.
            s, first, _ = units[u]
            if not first or s in slot_tiles:
                return
            w_sb = inp.tile([64, 128], dt.bfloat16, tag="w", name=f"w{s}")
            # sync queue: the K-side weight product gates the whole exp chain
            nc.sync.dma_start(w_sb[:], w_d[s])
            kt_sb = inp.tile([64, L_], dt.bfloat16, tag="kt", name=f"kt{s}")
            vt_sb = inp.tile([64, L_], dt.bfloat16, tag="vt", name=f"vt{s}")
            slot_tiles[s] = [w_sb, None, None, vt_sb, kt_sb]

        kprojd = {}

        def slot_kproj(u, j_hi=4):
            # staged on unit 0: its first score chunk only needs ktT cols
            # 0:256, so j0 (+copy) is emitted first, scores next, j1-3 after.
            s, first, _ = units[u]
            done = kprojd.get(s, 0)
            if not first or done >= j_hi:
                return
            kprojd[s] = j_hi
            w_sb, _, _, _, kt_sb = slot_tiles[s]
            if done == 0:
                slot_tiles[s][1] = proj.tile([64, L_], dt.bfloat16,
                                             tag="ktT", name=f"ktT{s}")
            ktT = slot_tiles[s][1]
            for j in range(done, j_hi):
                kps = ps_p.tile([64, 512], dt.float32, tag="pp", name=f"kps{s}_{j}")
                nc.tensor.matmul(kps[:], w_sb[:, 0:64],
                                 kt_sb[:, j * 512:(j + 1) * 512],
                                 start=True, stop=True)
                # all copies on DVE: ScalarE stays dedicated to exp
                nc.vector.tensor_copy(ktT[:, j * 512:(j + 1) * 512], kps[:])

        def slot_kv_dma(u, phase=2):
            # kt split so the j0 projection (all unit-0 startup needs) only
            # waits on the first 512 columns; unit 0's qt DMA is issued
            # between the halves so it isn't queued behind the big transfer
            s, first, _ = units[u]
            if not first:
                return
            done = kv_dmad.get(s, 0)
            kt_sb, vt_sb = slot_tiles[s][4], slot_tiles[s][3]
            if done < 1 and phase >= 0:
                nc.sync.dma_start(kt_sb[:, 0:512], kt_d[s][:, 0:512])
                kv_dmad[s] = 1
            if kv_dmad[s] < 2 and phase >= 1:
                nc.sync.dma_start(kt_sb[:, 512:], kt_d[s][:, 512:])
                nc.gpsimd.dma_start(vt_sb[:], vt_d[s])
                kv_dmad[s] = 2

        def slot_v_prologue(u):
            s, first, _ = units[u]
            if not first or slot_tiles[s][2] is not None:
                return
            w_sb, vt_sb = slot_tiles[s][0], slot_tiles[s][3]
            # v projection into [k=128, 16, 65] layout (col 64 = ones)
            v_sb = proj.tile([128, KT, 65], dt.bfloat16, tag="v_sb")
            for half in range(2):
                vps = ps_p.tile([128, 8 * 64], dt.float32, tag="pp")
                for j in range(8):
                    t = half * 8 + j
                    nc.tensor.matmul(vps[:, j * 64:(j + 1) * 64],
                                     vt_sb[:, t * 128:(t + 1) * 128],
                                     w_sb[:, 64:128], start=True, stop=True)
                nc.vector.tensor_copy(
                    v_sb[:, half * 8:(half + 1) * 8, 0:64],
                    vps[:].rearrange("p (t d) -> p t d", t=8))
            nc.vector.memset(v_sb[:, :, 64], 1.0)
            slot_tiles[s][2] = v_sb

        def unit_prologue(u):
            s, _, nq = units[u]
            d = st[u]
            d["init"] = True
            d["s"] = s
            d["chunks"] = _chunk_plan(nq, first=(u == 0))
            d["offs"] = [sum(d["chunks"][:i]) for i in range(len(d["chunks"]) + 1)]
            d["nq"] = nq
            qt_sb = inp.tile([64, nq], dt.bfloat16, tag="qt", name=f"qt{u}")
            nc.sync.dma_start(qt_sb[:], qt_d[u][:, 0:nq])
            d["qTh"] = qt_sb
            d["sps"] = [None] * len(d["chunks"])
            d["ex"] = [None] * len(d["chunks"])
            d["next_c"] = 0

        def s_chunk(u, c):
            d = st[u]
            cl, nq = d["chunks"][c], d["nq"]
            pb = _per_bank(nq)
            nb = (cl + pb - 1) // pb
            sps = ps_s.tile([128, nb, pb, nq], dt.float32, tag="ps",
                            name=f"sps{u}_{c}",
                            padded_shape=[None, None, None, 512 // pb])
            for j in range(cl):
                t = d["offs"][c] + j
                nc.tensor.matmul(sps[:, j // pb, j % pb, :],
                                 slot_tiles[d["s"]][1][:, t * 128:(t + 1) * 128],
                                 d["qTh"][:], start=True, stop=True)
            d["sps"][c] = sps

        def e_chunk(u, c):
            d = st[u]
            cl, nq = d["chunks"][c], d["nq"]
            pb = _per_bank(nq)
            nb = (cl + pb - 1) // pb
            ex = expp.tile([128, nb, pb, nq], dt.bfloat16, tag="ex", name=f"ex{u}_{c}")
            nc.scalar.activation(ex[:], d["sps"][c][:],
                                 mybir.ActivationFunctionType.Exp)
            d["ex"][c] = ex

        def av_chunk(u, c):
            # AV in O[q, d] orientation: lhsT = exp-scores [k, q-subtile],
            # rhs = v_sb [k, 65] (col 64 = ones -> denominators). Output free
            # dim is 65, so PE cost per k-tile is 65*NSUB cycles instead of
            # nq -- about half of the [d, q] orientation for nq=512. All
            # NSUB accumulation regions share one PSUM bank.
            d = st[u]
            nq = d["nq"]
            v_sb = slot_tiles[d["s"]][2]
            nsub = (nq + 127) // 128
            if c == 0:
                d["av"] = ps_a.tile([128, nsub, 65], dt.float32, tag="pa",
                                    name=f"av{u}")
            pb = _per_bank(nq)
            for j in range(d["chunks"][c]):
                t = d["offs"][c] + j
                for s in range(nsub):
                    w = min(128, nq - s * 128)
                    # all NSUB accumulation regions share one PSUM bank; a
                    # start=True matmul zeroes the whole bank, so only the
                    # very first matmul of the unit starts the group and only
                    # the very last stops it.
                    nc.tensor.matmul(
                        d["av"][0:w, s, :],
                        d["ex"][c][:, j // pb, j % pb, s * 128:s * 128 + w],
                        v_sb[:, t, :],
                        start=(t == 0 and s == 0),
                        stop=(t == KT - 1 and s == nsub - 1),
                        skip_group_check=True)

        def epilogue(u):
            # normalize per q-row: reciprocal of the ones-column, then one
            # per-partition tensor_scalar multiply per 128-row subtile.
            # Output lands directly in [q, d] layout (no host transpose).
            d = st[u]
            nq = d["nq"]
            nsub = (nq + 127) // 128
            rcp = ob.tile([128, nsub], dt.float32, tag="rs", name=f"rs{u}")
            ot = ob.tile([128, nsub, 64], dt.bfloat16, tag="ot", name=f"ot{u}")
            for s in range(nsub):
                w = min(128, nq - s * 128)
                nc.vector.reciprocal(rcp[0:w, s:s + 1], d["av"][0:w, s, 64:65])
                with nc.allow_low_precision(reason="final output cast; 2e-2 rel-err budget"):
                    nc.vector.tensor_scalar_mul(ot[0:w, s, :],
                                                d["av"][0:w, s, 0:64],
                                                rcp[0:w, s:s + 1])
            nc.sync.dma_start(out_d[u][:, 0:nsub * 64],
                              ot[:].rearrange("p a b -> p (a b)"))
            st[u].clear()

        def mean_block():
            # masked-head rank-1 content: (sum_k V_seq) @ (WV/2048). The
            # k-sum runs on PE (V in k-major layout x ones column) so DVE
            # stays free for the pipeline's copies and epilogues.
            wvm_sb = single.tile([64, H_ * 64], dt.float32)
            nc.sync.dma_start(wvm_sb[:], wvm_d[:])
            ones1 = single.tile([128, 1], dt.bfloat16)
            nc.vector.memset(ones1[:], 1.0)
            mvp = ps_p.tile([64, B_], dt.float32, tag="pp", name="mvp")
            vkm_sb = single.tile([128, B_, KT, 64], dt.bfloat16)
            for b in range(B_):
                nc.sync.dma_start(vkm_sb[:, b], vkm_d[b])
            for b in range(B_):
                for t in range(KT):
                    nc.tensor.matmul(mvp[:, b:b + 1], vkm_sb[:, b, t, :],
                                     ones1[:],
                                     start=(b == 0 and t == 0),
                                     stop=(b == B_ - 1 and t == KT - 1),
                                     skip_group_check=True)
            mvt = single.tile([64, B_], dt.float32)
            nc.vector.tensor_copy(mvt[:], mvp[:])
            mo_sb = single.tile([128, 8, B_], dt.float32)
            mps = ps_p.tile([128, 8, B_], dt.float32, tag="pp", name="mps")
            for c in range(8):
                nc.tensor.matmul(mps[:, c, :], wvm_sb[:, c * 128:(c + 1) * 128],
                                 mvt[:], start=(c == 0), stop=(c == 7),
                                 skip_group_check=True)
            nc.vector.tensor_copy(mo_sb[:], mps[:])
            nc.sync.dma_start(mo_d[:], mo_sb[:])

        # software pipeline across chunk-units: the next unit's prologue and
        # first TWO score chunks are emitted before this unit's AV tail and
        # epilogue so ScalarE never starves at unit boundaries.
        def emit_se(u1, c):
            if u1 >= NU or not st[u1].get("init"):
                return
            d = st[u1]
            if c >= len(d["chunks"]) or c < d["next_c"]:
                return
            s_chunk(u1, c)
            e_chunk(u1, c)
            d["next_c"] = c + 1

        def full_prologue(u1):
            # per-slot projections run just-in-time (1.5-2 units ahead) in
            # their own PSUM bank so they never serialize the score ring
            if u1 >= NU or st[u1].get("init"):
                return
            slot_k_prologue(u1)
            slot_kv_dma(u1)
            slot_kproj(u1)
            slot_v_prologue(u1)
            unit_prologue(u1)

        slot_k_prologue(0)
        slot_kv_dma(0, phase=0)
        unit_prologue(0)
        slot_kv_dma(0, phase=1)
        slot_kproj(0, j_hi=1)
        emit_se(0, 0)
        slot_kproj(0)
        slot_v_prologue(0)
        # issue every other slot's K/V DMAs now (cheap; transfers overlap
        # unit 0's compute) -- projections stay just-in-time
        first_unit = {}
        for i, (s, first, _) in enumerate(units):
            if first:
                first_unit[s] = i
        for s in range(1, S):
            slot_k_prologue(first_unit[s])
            slot_kv_dma(first_unit[s])
        full_prologue(1)
        for u in range(NU):
            nch = len(st[u]["chunks"])
            for c in range(nch):
                emit_se(u, c + 1)
                if c == 0:
                    full_prologue(u + 2)
                if c == max(0, nch - 2):
                    emit_se(u + 1, 0)
                av_chunk(u, c)
                if c == nch - 1:
                    emit_se(u + 1, 1)
            epilogue(u)
            if u == max(0, NU // 2 - 1):
                mean_block()

    nc.compile()
    return nc


def _units_of(rows):
    out = []
    while rows > NQ:
        out.append(NQ)
        rows -= NQ
    out.append(int(rows))
    return tuple(out)


def _act_cost(R):
    """ScalarE-time proxy for a slot-size vector: exp elements + per-instr
    access latency + per-slot misc."""
    t = 0.0
    for rows in R:
        for w in _units_of(rows):
            cl = 3 * _per_bank(w)
            t += 16 * w * 0.8333 + 190 * ((KT + cl - 1) // cl)
        t += 200.0
    return t


def _cover(needs, R, limit=60000):
    """Cover each job (needs, descending) with pieces drawn from 8 instances
    of each slot size R[j]. Returns per-job lists of slot indices, or None."""
    J = len(R)
    nodes = [0]

    def combos(need, avail):
        idxs = [j for j in range(J) if avail[j] > 0]
        out = []
        for k in range(1, 5):
            for ms in itertools.combinations_with_replacement(idxs, k):
                cnt = {}
                ok = True
                for j in ms:
                    cnt[j] = cnt.get(j, 0) + 1
                    if cnt[j] > avail[j]:
                        ok = False
                        break
                if not ok:
                    continue
                ssum = sum(R[j] for j in ms)
                if ssum >= need:
                    out.append((ssum - need, k, ms))
        out.sort(key=lambda x: (x[0], x[1]))
        seen, res = set(), []
        for waste, k, ms in out:
            key = tuple(sorted(R[j] for j in ms))
            if key in seen:
                continue
            seen.add(key)
            res.append(ms)
            if len(res) >= 6:
                break
        return res

    def rec(i, avail):
        nodes[0] += 1
        if nodes[0] > limit:
            return None
        if i == len(needs):
            return []
        for ms in combos(needs[i], avail):
            av2 = list(avail)
            for j in ms:
                av2[j] -= 1
            sub = rec(i + 1, av2)
            if sub is not None:
                return [list(ms)] + sub
        return None

    return rec(0, [N_CORES] * J)


def _plan(q_len, v_len, B, L, H):
    """Pack unmasked-head work into uniform per-core slots, splitting heads
    across cores to balance rows (the exp on ScalarE scales with per-core
    rows).

    Returns (struct, assign): struct[s] = tuple of unit q-widths for slot s;
    assign[(core, s)] = (b, h, off) or None, where the piece covers rows
    [off, off + sum(struct[s])) of head (b, h)."""
    jobs = []
    for b in range(B):
        nq = min(max(q_len[b], 0), L)
        nh = min(max(v_len[b], 0), H)
        if nq <= 0:
            continue
        for h in range(nh):
            jobs.append((nq, b, h))
    if not jobs:
        jobs = [(64, 0, 0)]
    jobs.sort(key=lambda x: -x[0])
    needs = [j[0] for j in jobs]

    # guaranteed-feasible fallback: deal whole heads into columns
    n_slots = max(1, (len(jobs) + N_CORES - 1) // N_CORES)
    cands = [tuple(needs[s * N_CORES] for s in range(n_slots))]
    # two-piece/LP family: (B, ceil(A/2), s2, s3) — splits the largest heads
    # in half and covers the remainders with two small slot sizes
    uniq = sorted(set(needs), reverse=True)
    if len(jobs) <= 24 and len(uniq) >= 1:
        A = uniq[0]
        halfA = -(-A // 2)
        for Bn in (uniq[1:2] or [A]):
            for s3 in range(96, min(513, Bn)):
                s2a = -(-(Bn - 3 * s3) // 2)
                s2b = -(-(A - halfA - s3) // 2)
                s2 = max(s2a, s2b, s3, 96)
                if s2 <= 512:
                    cands.append((Bn, halfA, s2, s3))
            cands.append((Bn, halfA, halfA))
            cands.append((A, Bn, 512))
    cands.sort(key=_act_cost)

    best = None
    for R in cands:
        if best is not None and _act_cost(R) >= best[0]:
            continue
        cover = _cover(needs, R)
        if cover is not None:
            best = (_act_cost(R), R, cover)
    _, R, cover = best

    # order slots descending so the round-robin tail lands on a small unit
    order = sorted(range(len(R)), key=lambda j: -R[j])
    inv = {j: i for i, j in enumerate(order)}
    slot_pieces = [[] for _ in range(len(R))]
    for (nq, b, h), ms in zip(jobs, cover):
        acc = 0
        for j in sorted(ms, key=lambda j: -R[j]):
            off = max(0, min(acc, L - R[j]))
            slot_pieces[inv[j]].append((b, h, int(off)))
            acc += R[j]
    struct = tuple(_units_of(R[j]) for j in order)
    assign = {}
    for s in range(len(R)):
        for c in range(N_CORES):
            assign[(c, s)] = (slot_pieces[s][c]
                              if c < len(slot_pieces[s]) else None)
    return struct, assign


def kernel(Q_seq, K_seq, V_seq, WQ, WK, WV, Q_len, V_len):
    Q_seq = np.asarray(Q_seq, dtype=np.float32)
    K_seq = np.asarray(K_seq, dtype=np.float32)
    V_seq = np.asarray(V_seq, dtype=np.float32)
    WQ = np.asarray(WQ, dtype=np.float32)
    WK = np.asarray(WK, dtype=np.float32)
    WV = np.asarray(WV, dtype=np.float32)
    q_len = [int(x) for x in np.asarray(Q_len).reshape(-1)]
    v_len = [int(x) for x in np.asarray(V_len).reshape(-1)]
    B, L, d = Q_seq.shape
    H = WQ.shape[1] // d
    scale = 1.0 / math.sqrt(d)

    struct, assign = _plan(q_len, v_len, B, L, H)
    S = len(struct)
    order = _unit_order(struct)
    row_of = {sr: i for i, sr in enumerate(order)}
    NU = len(order)

    if struct not in _cache:
        _cache[struct] = _build(struct)
    nc = _cache[struct]

    # host-side shard prep (transposes, bf16 casts, weight slicing)
    KTb = [np.ascontiguousarray(K_seq[b].T).astype(BF16) for b in range(B)]
    VTb = [np.ascontiguousarray(V_seq[b].T).astype(BF16) for b in range(B)]
    QT = [np.ascontiguousarray(Q_seq[b].T).astype(BF16) for b in range(B)]
    vkm = np.ascontiguousarray(
        V_seq.reshape(B, KT, 128, d).transpose(0, 2, 1, 3)
    ).reshape(B, 128, KT * d).astype(BF16)
    wvm = (WV / float(L)).astype(np.float32)

    in_maps = []
    for c in range(N_CORES):
        qt = np.zeros((NU, 64, NQ), dtype=BF16)
        kt = np.zeros((S, 64, L), dtype=BF16)
        vt = np.zeros((S, 64, L), dtype=BF16)
        w = np.zeros((S, 64, 128), dtype=BF16)
        for s in range(S):
            job = assign[(c, s)]
            if job is None:
                continue
            b, h, off = job
            kt[s] = KTb[b]
            vt[s] = VTb[b]
            wq_h = WQ[:, h * d:(h + 1) * d]
            wk_h = WK[:, h * d:(h + 1) * d]
            w[s, :, 0:64] = (wk_h @ wq_h.T * scale).astype(BF16)
            w[s, :, 64:128] = WV[:, h * d:(h + 1) * d].astype(BF16)
            start = 0
            for r, nqw in enumerate(struct[s]):
                q0 = min(off + start, L - nqw)
                qt[row_of[(s, r)], :, 0:nqw] = QT[b][:, q0:q0 + nqw]
                start += nqw
        in_maps.append({"qt": qt, "kt": kt, "vt": vt, "w": w,
                        "vkm": vkm, "wvm": wvm})

    global _last_in_maps
    _last_in_maps = in_maps
    res = run_bass_kernel_spmd(nc, in_maps, core_ids=list(range(N_CORES)))
    results = res.results

    # gather
    out = np.zeros((B, L, H * d), dtype=np.float32)
    mo = results[0]["meanout"]  # [128, 8, B]
    mean_proj = np.transpose(mo, (2, 1, 0)).reshape(B, H * d)  # [B, H*d]
    for b in range(B):
        nq = min(max(q_len[b], 0), L)
        nh = min(max(v_len[b], 0), H)
        if nq > 0 and nh < H:
            out[b, :nq, nh * d:] = mean_proj[b, nh * d:][None, :]
    for (c, s), job in assign.items():
        if job is None:
            continue
        b, h, off = job
        nq = min(max(q_len[b], 0), L)
        start = 0
        for r, nqw in enumerate(struct[s]):
            q0 = min(off + start, L - nqw)
            start += nqw
            blk = results[c]["out"][row_of[(s, r)]].reshape(128, 4, 64)
            for sub in range((nqw + 127) // 128):
                w = min(128, nqw - sub * 128)
                lo = q0 + sub * 128
                hi = min(lo + w, nq)
                if hi <= lo:
                    continue
                out[b, lo:hi, h * d:(h + 1) * d] = \
                    blk[0:hi - lo, sub, :].astype(np.float32)
    return out

